# revision 13
# baseline (speedup 1.0000x reference)
"""EnsembleRAM (WNN) forward kernel for 8 Trainium2 NeuronCores.

Strategy (model-parallel over RAMs):
  - Core c owns RAMs {2c, 2c+1} = 2048 neurons x all 1024 samples.
  - Phase 1: dma_gather pulls, per neuron tap, the x-bit column (1024
    samples, fp8) out of the transposed input xT[4096, 1024] in HBM.
    A PE matmul with static power-of-two weights sums the 8 taps of each
    neuron into its 8-bit table address (exact in fp32 PSUM).
  - Phase 2: each neuron's 256-bit truth table is packed host-side into
    16 uint16 words (word h = table[16h:16h+16]).  A 4-level binary
    select tree on DVE (copy_predicated, driven by addr bits 4..7) picks
    word16 = W[n, addr>>4]; a per-element variable shift by (addr & 15)
    extracts the looked-up bit.
  - Phase 3: per-core partial votes (2 RAMs) are ReduceScatter-summed
    across the 8 cores; each core thresholds its 128-output slice.

Everything is integer-exact: fp8/fp32 hold {0,1,2,...,255} exactly and
word values <= 65535 are exact in fp32.
"""

import numpy as np
from contextlib import ExitStack

# Problem constants (hardcoded; kernel.py must be self-contained)
R, O, K, T = 16, 1024, 8, 256
B, IB = 1024, 4096
N_CORES = 8
RPC = R // N_CORES          # RAMs per core = 2
NPC = RPC * O               # neurons per core = 2048
ROUNDS = NPC // 128         # 16 rounds of 128 neurons
GROUPS = 8                  # matmul groups per round (16 neurons each)

_BUILT = None


def _build_bass():
    import concourse.bacc as bacc
    import concourse.tile as tile
    from concourse import mybir

    dt = mybir.dt
    nc = bacc.Bacc(
        "TRN2",
        target_bir_lowering=False,
        debug=False,
        num_devices=N_CORES,
    )

    xT = nc.declare_dram_parameter("xT", [IB, B], dt.float8e4, False)
    gidx = nc.declare_dram_parameter("gidx", [128, 64 * ROUNDS], dt.int16, False)
    shw = nc.declare_dram_parameter("shw", [128, 64], dt.float8e4, False)
    # wsel[p, 16*ri + j] : j in 0..7 -> E_j (W[2j]), j in 8..15 -> D_{j-8} (W[2j+1]-W[2j])
    wsel = nc.declare_dram_parameter("wsel", [128, 16 * ROUNDS], dt.float32, False)
    res = nc.declare_dram_parameter("res", [128, B], dt.int32, True)

    votes_dram = nc.dram_tensor("votes_dram", [O, B], dt.float32)
    rs_out = nc.dram_tensor("rs_out", [128, B], dt.float32)

    aop = mybir.AluOpType

    with tile.TileContext(nc) as tc:
        with (
            tc.tile_pool(name="const", bufs=1) as cpool,
            tc.tile_pool(name="gat", bufs=2) as gpool,
            tc.tile_pool(name="work", bufs=2) as wpool,
            tc.tile_pool(name="sel", bufs=2) as spool,
            tc.tile_pool(name="vot", bufs=1) as vpool,
            tc.tile_pool(name="fin", bufs=2) as fpool,
            tc.tile_pool(name="ps", bufs=2, space="PSUM") as ppool,
        ):
            gidx_t = cpool.tile([128, 64 * ROUNDS], dt.int16)
            nc.sync.dma_start(out=gidx_t[:], in_=gidx[:])
            shw_t = cpool.tile([128, 64], dt.float8e4)
            nc.sync.dma_start(out=shw_t[:], in_=shw[:])
            wsel_t = cpool.tile([128, 16 * ROUNDS], dt.float32)
            nc.sync.dma_start(out=wsel_t[:], in_=wsel[:])

            votes = []
            for i in range(8):
                v = vpool.tile([128, B], dt.uint16, tag=f"v{i}")
                nc.vector.memset(v[:], 0)
                votes.append(v)

            for ri in range(ROUNDS):
                # ---- phase 1: gather 1024 x-bit rows (128 neurons x 8 taps)
                G = gpool.tile([128, GROUPS, B], dt.float8e4, tag="G")
                nc.gpsimd.dma_gather(
                    out_ap=G[:],
                    in_ap=xT[:],
                    idxs_ap=gidx_t[:, 64 * ri : 64 * (ri + 1)],
                    num_idxs=1024,
                    num_idxs_reg=1024,
                    elem_size=B,
                )
                # ---- phase 1b: tap-sum matmuls -> addr in PSUM (fp32, exact)
                # chunk q = 2k + h holds (neuron-slice k, tap-half h); the two
                # tap-halves accumulate into the same 32-partition PSUM slice.
                pa = ppool.tile([128, 512], dt.float32, tag="pa")
                pb = ppool.tile([128, 512], dt.float32, tag="pb")
                for k in range(4):
                    for half, ps in ((0, pa), (1, pb)):
                        for h in range(2):
                            nc.tensor.matmul(
                                ps[32 * k : 32 * (k + 1), :],
                                lhsT=shw_t[:, 32 * h : 32 * (h + 1)],
                                rhs=G[:, 2 * k + h, 512 * half : 512 * (half + 1)],
                                start=(h == 0),
                                stop=(h == 1),
                                tile_position=(0, 32 * k),
                            )
                # ---- phase 2: addr -> looked-up bit
                a = wpool.tile([128, B], dt.uint16, tag="a")
                nc.scalar.copy(out=a[:, 0:512], in_=pa[:])
                nc.scalar.copy(out=a[:, 512:1024], in_=pb[:])

                # b4 in {0, 16}; the host pre-divides the D scalars by 16
                b4 = wpool.tile([128, B], dt.uint16, tag="b4")
                nc.vector.tensor_scalar(b4[:], a[:], 16, None, aop.bitwise_and)
                m5 = wpool.tile([128, B], dt.uint16, tag="m5")
                nc.vector.tensor_scalar(m5[:], a[:], 32, None, aop.bitwise_and)
                m6 = wpool.tile([128, B], dt.uint16, tag="m6")
                nc.vector.tensor_scalar(m6[:], a[:], 64, None, aop.bitwise_and)
                m7 = wpool.tile([128, B], dt.uint16, tag="m7")
                nc.vector.tensor_scalar(m7[:], a[:], 128, None, aop.bitwise_and)
                lo = wpool.tile([128, B], dt.uint16, tag="lo")
                nc.vector.tensor_scalar(lo[:], a[:], 15, None, aop.bitwise_and)

                s = []
                for j in range(8):
                    sj = spool.tile([128, B], dt.uint16, tag=f"s{j}")
                    dj = wsel_t[:, 16 * ri + 8 + j : 16 * ri + 9 + j]
                    ej = wsel_t[:, 16 * ri + j : 16 * ri + j + 1]
                    nc.vector.tensor_scalar(
                        sj[:], b4[:], dj, ej, aop.mult, aop.add
                    )
                    s.append(sj)
                for k in range(4):
                    nc.vector.copy_predicated(s[2 * k][:], m5[:], s[2 * k + 1][:])
                nc.vector.copy_predicated(s[0][:], m6[:], s[2][:])
                nc.vector.copy_predicated(s[4][:], m6[:], s[6][:])
                nc.vector.copy_predicated(s[0][:], m7[:], s[4][:])

                sh = wpool.tile([128, B], dt.uint16, tag="sh")
                nc.vector.tensor_tensor(
                    out=sh[:], in0=s[0][:], in1=lo[:], op=aop.logical_shift_right
                )
                bit = wpool.tile([128, B], dt.uint16, tag="bit")
                nc.vector.tensor_scalar(bit[:], sh[:], 1, None, aop.bitwise_and)
                vt = votes[ri % 8]
                nc.vector.tensor_tensor(
                    out=vt[:], in0=vt[:], in1=bit[:], op=aop.add
                )

            # ---- phase 3: votes -> DRAM, ReduceScatter, threshold
            for i in range(8):
                vf = fpool.tile([128, B], dt.float32, tag="vf")
                nc.vector.tensor_copy(out=vf[:], in_=votes[i][:])
                nc.sync.dma_start(
                    out=votes_dram[128 * i : 128 * (i + 1), :], in_=vf[:]
                )
            nc.gpsimd.collective_compute(
                "ReduceScatter",
                mybir.AluOpType.add,
                replica_groups=[list(range(N_CORES))],
                ins=[votes_dram[:]],
                outs=[rs_out[:]],
            )
            rsb = fpool.tile([128, B], dt.float32, tag="rsb")
            nc.sync.dma_start(out=rsb[:], in_=rs_out[:])
            thr = fpool.tile([128, B], dt.int32, tag="thr")
            nc.vector.tensor_scalar(
                thr[:], rsb[:], float(R // 2), None, mybir.AluOpType.is_gt
            )
            nc.sync.dma_start(out=res[:], in_=thr[:])

    nc.compile()
    return nc


def _host_tables(projections, connections, memory):
    """Per-core gather indices + packed table words (x-independent)."""
    from concourse import mybir

    dt = mybir.dt
    fp8 = np.dtype(mybir.dt.np(dt.float8e4))

    proj = np.asarray(projections)
    conn = np.asarray(connections)
    mem = np.asarray(memory)

    # absolute input-bit index per (ram, neuron, tap)
    full_idx = proj[np.arange(R)[:, None, None], conn]  # [R, O, K] int

    # packed 16-bit table words: W16[r, o, h] = sum_l mem[r,o,16h+l] << l
    mem_i = mem.astype(np.int64).reshape(R, O, 16, 16)
    weights = (1 << np.arange(16, dtype=np.int64))
    W16 = (mem_i * weights).sum(axis=-1)  # [R, O, 16]

    # shw[p=(t' + 4u), u] = 2^t' (cols 0..31, tap-half A) / 2^(t'+4) (cols 32..63)
    shw = np.zeros((128, 64), np.float64)
    for u in range(32):
        for tp in range(4):
            shw[tp + 4 * u, u] = float(1 << tp)
            shw[tp + 4 * u, 32 + u] = float(1 << (tp + 4))
    shw = shw.astype(fp8)
    gidx_all, wsel_all = [], []

    for c in range(N_CORES):
        gidx_c = np.zeros((128, 64 * ROUNDS), np.int16)
        wsel_c = np.zeros((128, 16 * ROUNDS), np.float32)
        for ri in range(ROUNDS):
            r = 2 * c + (ri // 8)
            oblk = ri % 8
            # gather index list: chunk q = 2k + h, p = t' + 4u
            # neuron o = oblk*128 + 32k + u, tap t = 4h + t'
            idx_flat = np.zeros(1024, np.int16)
            for k in range(4):
                for h in range(2):
                    q = 2 * k + h
                    for u in range(32):
                        o = oblk * 128 + 32 * k + u
                        for tp in range(4):
                            idx_flat[q * 128 + tp + 4 * u] = full_idx[r, o, 4 * h + tp]
            # wrap into 16 partitions: idx_flat[i] -> [i % 16, i // 16]
            wrapped = idx_flat.reshape(64, 16).T  # [16, 64]
            for k in range(8):
                gidx_c[16 * k : 16 * (k + 1), 64 * ri : 64 * (ri + 1)] = wrapped
            # select-tree scalars for the round's 128 neurons
            o_ids = oblk * 128 + np.arange(128)
            Wr = W16[r, o_ids, :]  # [128, 16]
            for j in range(8):
                wsel_c[:, 16 * ri + j] = Wr[:, 2 * j].astype(np.float32)
                wsel_c[:, 16 * ri + 8 + j] = (
                    (Wr[:, 2 * j + 1] - Wr[:, 2 * j]).astype(np.float32) / 16.0
                )
        gidx_all.append(gidx_c)
        wsel_all.append(wsel_c)
    return gidx_all, wsel_all, shw


def kernel(x, projections, connections, memory):
    global _BUILT
    from concourse import mybir
    from concourse.bass_utils import run_bass_kernel_spmd

    dt = mybir.dt
    fp8 = np.dtype(mybir.dt.np(dt.float8e4))

    if _BUILT is None:
        _BUILT = _build_bass()
    nc = _BUILT

    gidx_all, wsel_all, shw = _host_tables(projections, connections, memory)
    xT = np.ascontiguousarray(np.asarray(x).T).astype(fp8)  # [4096, 1024]

    in_maps = [
        {"xT": xT, "gidx": gidx_all[c], "shw": shw, "wsel": wsel_all[c]}
        for c in range(N_CORES)
    ]
    out = run_bass_kernel_spmd(nc, in_maps, list(range(N_CORES)))
    results = out.results

    full = np.zeros((B, O), np.int32)
    for c in range(N_CORES):
        full[:, 128 * c : 128 * (c + 1)] = results[c]["res"].T
    return full


if __name__ == "__main__":
    import reference

    inputs = reference.setup_inputs()
    expected = np.asarray(reference.reference(**inputs))
    actual = kernel(**{k: np.asarray(v) for k, v in inputs.items()})
    err = np.abs(actual - expected).max()
    print("max abs err:", err)


# revision 19
# speedup vs baseline: 1.1998x; 1.1998x over previous
"""EnsembleRAM (WNN) forward kernel for 8 Trainium2 NeuronCores.

Strategy (model-parallel over RAMs):
  - Core c owns RAMs {2c, 2c+1} = 2048 neurons x all 1024 samples.
  - Phase 1: dma_gather pulls, per neuron tap, the x-bit column (1024
    samples, fp8) out of the transposed input xT[4096, 1024] in HBM.
    A PE matmul with static power-of-two weights sums the 8 taps of each
    neuron into its 8-bit table address (exact in fp32 PSUM).
  - Phase 2: each neuron's 256-bit truth table is packed host-side into
    16 uint16 words (word h = table[16h:16h+16]).  A 4-level binary
    select tree on DVE (copy_predicated, driven by addr bits 4..7) picks
    word16 = W[n, addr>>4]; a per-element variable shift by (addr & 15)
    extracts the looked-up bit.
  - Phase 3: per-core partial votes (2 RAMs) are ReduceScatter-summed
    across the 8 cores; each core thresholds its 128-output slice.

Everything is integer-exact: fp8/fp32 hold {0,1,2,...,255} exactly and
word values <= 65535 are exact in fp32.
"""

import numpy as np
from contextlib import ExitStack

# Problem constants (hardcoded; kernel.py must be self-contained)
R, O, K, T = 16, 1024, 8, 256
B, IB = 1024, 4096
N_CORES = 8
RPC = R // N_CORES          # RAMs per core = 2
NPC = RPC * O               # neurons per core = 2048
ROUNDS = NPC // 128         # 16 rounds of 128 neurons
GROUPS = 8                  # matmul groups per round (16 neurons each)

_BUILT = None


def _build_bass():
    import concourse.bacc as bacc
    import concourse.tile as tile
    from concourse import mybir

    dt = mybir.dt
    nc = bacc.Bacc(
        "TRN2",
        target_bir_lowering=False,
        debug=False,
        num_devices=N_CORES,
    )

    xT = nc.declare_dram_parameter("xT", [IB, B], dt.float8e4, False)
    gidx = nc.declare_dram_parameter("gidx", [128, 64 * ROUNDS], dt.int16, False)
    shw = nc.declare_dram_parameter("shw", [128, 64], dt.float8e4, False)
    # wsel[p, 16*ri + j] : j in 0..7 -> E_j (W[2j]), j in 8..15 -> D_{j-8} (W[2j+1]-W[2j])
    wsel = nc.declare_dram_parameter("wsel", [128, 16 * ROUNDS], dt.float32, False)
    res = nc.declare_dram_parameter("res", [128, B], dt.int32, True)

    votes_dram = nc.dram_tensor("votes_dram", [O, B], dt.bfloat16)
    rs_out = nc.dram_tensor("rs_out", [128, B], dt.bfloat16)

    aop = mybir.AluOpType

    with tile.TileContext(nc) as tc:
        with (
            tc.tile_pool(name="const", bufs=1) as cpool,
            tc.tile_pool(name="gat", bufs=2) as gpool,
            tc.tile_pool(name="work", bufs=2) as wpool,
            tc.tile_pool(name="sel", bufs=2) as spool,
            tc.tile_pool(name="vot", bufs=1) as vpool,
            tc.tile_pool(name="fin", bufs=2) as fpool,
            tc.tile_pool(name="ps", bufs=2, space="PSUM") as ppool,
        ):
            gidx_t = cpool.tile([128, 64 * ROUNDS], dt.int16)
            nc.sync.dma_start(out=gidx_t[:], in_=gidx[:])
            shw_t = cpool.tile([128, 64], dt.float8e4)
            nc.sync.dma_start(out=shw_t[:], in_=shw[:])
            wsel_t = cpool.tile([128, 16 * ROUNDS], dt.float32)
            nc.sync.dma_start(out=wsel_t[:], in_=wsel[:])

            votes = []
            for i in range(8):
                v = vpool.tile([128, B], dt.uint16, tag=f"v{i}")
                nc.vector.memset(v[:], 0)
                votes.append(v)

            for ri in range(ROUNDS):
                # ---- phase 1: gather 1024 x-bit rows (128 neurons x 8 taps)
                G = gpool.tile([128, GROUPS, B], dt.float8e4, tag="G")
                nc.gpsimd.dma_gather(
                    out_ap=G[:],
                    in_ap=xT[:],
                    idxs_ap=gidx_t[:, 64 * ri : 64 * (ri + 1)],
                    num_idxs=1024,
                    num_idxs_reg=1024,
                    elem_size=B,
                )
                # ---- phase 1b: tap-sum matmuls -> addr in PSUM (fp32, exact)
                # chunk q = 2k + h holds (neuron-slice k, tap-half h); the two
                # tap-halves accumulate into the same 32-partition PSUM slice.
                pa = ppool.tile([128, 512], dt.float32, tag="pa")
                pb = ppool.tile([128, 512], dt.float32, tag="pb")
                for k in range(4):
                    for half, ps in ((0, pa), (1, pb)):
                        for h in range(2):
                            nc.tensor.matmul(
                                ps[32 * k : 32 * (k + 1), :],
                                lhsT=shw_t[:, 32 * h : 32 * (h + 1)],
                                rhs=G[:, 2 * k + h, 512 * half : 512 * (half + 1)],
                                start=(h == 0),
                                stop=(h == 1),
                                tile_position=(0, 32 * k),
                            )
                # ---- phase 2: addr -> looked-up bit
                a = wpool.tile([128, B], dt.uint16, tag="a")
                nc.scalar.copy(out=a[:, 0:512], in_=pa[:])
                nc.scalar.copy(out=a[:, 512:1024], in_=pb[:])

                # b4 in {0, 16}; the host pre-divides the D scalars by 16
                b4 = wpool.tile([128, B], dt.uint16, tag="b4")
                nc.vector.tensor_scalar(b4[:], a[:], 16, None, aop.bitwise_and)
                m5 = wpool.tile([128, B], dt.uint16, tag="m5")
                nc.vector.tensor_scalar(m5[:], a[:], 32, None, aop.bitwise_and)
                m6 = wpool.tile([128, B], dt.uint16, tag="m6")
                nc.vector.tensor_scalar(m6[:], a[:], 64, None, aop.bitwise_and)
                m7 = wpool.tile([128, B], dt.uint16, tag="m7")
                nc.vector.tensor_scalar(m7[:], a[:], 128, None, aop.bitwise_and)
                lo = wpool.tile([128, B], dt.uint16, tag="lo")
                nc.vector.tensor_scalar(lo[:], a[:], 15, None, aop.bitwise_and)

                # select-tree leaves on the (otherwise idle) Act engine:
                # s_j = Identity(b4 * D_j + E_j), exact in fp32
                s = []
                for j in range(8):
                    sj = spool.tile([128, B], dt.uint16, tag=f"s{j}")
                    dj = wsel_t[:, 16 * ri + 8 + j : 16 * ri + 9 + j]
                    ej = wsel_t[:, 16 * ri + j : 16 * ri + j + 1]
                    nc.scalar.activation(
                        sj[:], b4[:], mybir.ActivationFunctionType.Identity,
                        bias=ej, scale=dj,
                    )
                    s.append(sj)
                for k in range(4):
                    nc.vector.copy_predicated(s[2 * k][:], m5[:], s[2 * k + 1][:])
                nc.vector.copy_predicated(s[0][:], m6[:], s[2][:])
                nc.vector.copy_predicated(s[4][:], m6[:], s[6][:])
                nc.vector.copy_predicated(s[0][:], m7[:], s[4][:])

                sh = wpool.tile([128, B], dt.uint16, tag="sh")
                nc.vector.tensor_tensor(
                    out=sh[:], in0=s[0][:], in1=lo[:], op=aop.logical_shift_right
                )
                bit = wpool.tile([128, B], dt.uint16, tag="bit")
                nc.vector.tensor_scalar(bit[:], sh[:], 1, None, aop.bitwise_and)
                vt = votes[ri % 8]
                nc.vector.tensor_tensor(
                    out=vt[:], in0=vt[:], in1=bit[:], op=aop.add
                )

            # ---- phase 3: votes -> DRAM, ReduceScatter, threshold
            for i in range(8):
                vf = fpool.tile([128, B], dt.bfloat16, tag="vf")
                nc.scalar.copy(out=vf[:], in_=votes[i][:])
                nc.sync.dma_start(
                    out=votes_dram[128 * i : 128 * (i + 1), :], in_=vf[:]
                )
            nc.gpsimd.collective_compute(
                "ReduceScatter",
                mybir.AluOpType.add,
                replica_groups=[list(range(N_CORES))],
                ins=[votes_dram[:]],
                outs=[rs_out[:]],
            )
            rsb = fpool.tile([128, B], dt.bfloat16, tag="rsb")
            nc.sync.dma_start(out=rsb[:], in_=rs_out[:])
            thr = fpool.tile([128, B], dt.int32, tag="thr")
            nc.vector.tensor_scalar(
                thr[:], rsb[:], float(R // 2), None, mybir.AluOpType.is_gt
            )
            nc.sync.dma_start(out=res[:], in_=thr[:])

    nc.compile()
    return nc


def _host_tables(projections, connections, memory):
    """Per-core gather indices + packed table words (x-independent)."""
    from concourse import mybir

    dt = mybir.dt
    fp8 = np.dtype(mybir.dt.np(dt.float8e4))

    proj = np.asarray(projections)
    conn = np.asarray(connections)
    mem = np.asarray(memory)

    # absolute input-bit index per (ram, neuron, tap)
    full_idx = proj[np.arange(R)[:, None, None], conn]  # [R, O, K] int

    # packed 16-bit table words: W16[r, o, h] = sum_l mem[r,o,16h+l] << l
    mem_i = mem.astype(np.int64).reshape(R, O, 16, 16)
    weights = (1 << np.arange(16, dtype=np.int64))
    W16 = (mem_i * weights).sum(axis=-1)  # [R, O, 16]

    # shw[p=(t' + 4u), u] = 2^t' (cols 0..31, tap-half A) / 2^(t'+4) (cols 32..63)
    shw = np.zeros((128, 64), np.float64)
    for u in range(32):
        for tp in range(4):
            shw[tp + 4 * u, u] = float(1 << tp)
            shw[tp + 4 * u, 32 + u] = float(1 << (tp + 4))
    shw = shw.astype(fp8)
    gidx_all, wsel_all = [], []

    for c in range(N_CORES):
        gidx_c = np.zeros((128, 64 * ROUNDS), np.int16)
        wsel_c = np.zeros((128, 16 * ROUNDS), np.float32)
        for ri in range(ROUNDS):
            r = 2 * c + (ri // 8)
            oblk = ri % 8
            # gather index list: chunk q = 2k + h, p = t' + 4u
            # neuron o = oblk*128 + 32k + u, tap t = 4h + t'
            idx_flat = np.zeros(1024, np.int16)
            for k in range(4):
                for h in range(2):
                    q = 2 * k + h
                    for u in range(32):
                        o = oblk * 128 + 32 * k + u
                        for tp in range(4):
                            idx_flat[q * 128 + tp + 4 * u] = full_idx[r, o, 4 * h + tp]
            # wrap into 16 partitions: idx_flat[i] -> [i % 16, i // 16]
            wrapped = idx_flat.reshape(64, 16).T  # [16, 64]
            for k in range(8):
                gidx_c[16 * k : 16 * (k + 1), 64 * ri : 64 * (ri + 1)] = wrapped
            # select-tree scalars for the round's 128 neurons
            o_ids = oblk * 128 + np.arange(128)
            Wr = W16[r, o_ids, :]  # [128, 16]
            for j in range(8):
                wsel_c[:, 16 * ri + j] = Wr[:, 2 * j].astype(np.float32)
                wsel_c[:, 16 * ri + 8 + j] = (
                    (Wr[:, 2 * j + 1] - Wr[:, 2 * j]).astype(np.float32) / 16.0
                )
        gidx_all.append(gidx_c)
        wsel_all.append(wsel_c)
    return gidx_all, wsel_all, shw


def kernel(x, projections, connections, memory):
    global _BUILT
    from concourse import mybir
    from concourse.bass_utils import run_bass_kernel_spmd

    dt = mybir.dt
    fp8 = np.dtype(mybir.dt.np(dt.float8e4))

    if _BUILT is None:
        _BUILT = _build_bass()
    nc = _BUILT

    gidx_all, wsel_all, shw = _host_tables(projections, connections, memory)
    xT = np.ascontiguousarray(np.asarray(x).T).astype(fp8)  # [4096, 1024]

    in_maps = [
        {"xT": xT, "gidx": gidx_all[c], "shw": shw, "wsel": wsel_all[c]}
        for c in range(N_CORES)
    ]
    out = run_bass_kernel_spmd(nc, in_maps, list(range(N_CORES)))
    results = out.results

    full = np.zeros((B, O), np.int32)
    for c in range(N_CORES):
        full[:, 128 * c : 128 * (c + 1)] = results[c]["res"].T
    return full


if __name__ == "__main__":
    import reference

    inputs = reference.setup_inputs()
    expected = np.asarray(reference.reference(**inputs))
    actual = kernel(**{k: np.asarray(v) for k, v in inputs.items()})
    err = np.abs(actual - expected).max()
    print("max abs err:", err)
